# revision 49
# baseline (speedup 1.0000x reference)
"""Multi-head attention (B=2, S=2048, E=1024, H=16, causal) on 8 Trainium2 cores.

Sharding: data-parallel over batch (2) x tensor-parallel over heads (4 groups
of 4 heads). Core i handles batch i//4, heads 4*(i%4) .. 4*(i%4)+3.
Each core computes Q/K/V projections for its 256 channels, causal
flash-attention for its 4 heads, and a partial output projection
(contribution of its channels to all 1024 output features). Partials are
summed across the 4 cores of each batch group (host-side).

Input DMAs run in consumption order, round-robin across the three
DMA-capable rings (sync/scalar/gpsimd); weights are pre-packed host-side
into SBUF layout so each is one big contiguous transfer (DMA cost is
descriptor-bound). Causal masking is column-trimmed: score matmuls, exp and
PV skip the fully-masked left columns of diagonal tiles (PV stops each
128-column group at its diagonal tile), and affine_select only covers the
masked+triangle region. Out-projection accumulates both c-chunks in PSUM
(one DVE add per tile) and stores bf16 partials (summed on host in f32).
"""
import contextlib

import numpy as np

import concourse.bass as bass
import concourse.tile as tile
from concourse import bacc, mybir
from concourse.bass_utils import run_bass_kernel_spmd

F32 = mybir.dt.float32
F32R = mybir.dt.float32r
BF16 = mybir.dt.bfloat16
import ml_dtypes
MM_DT = BF16
MM_NP = ml_dtypes.bfloat16
ActF = mybir.ActivationFunctionType
Alu = mybir.AluOpType

B, S, E = 2, 2048, 1024
H, DH = 16, 64
NCORES, TPW = 8, 4          # 8 cores, 4-way tensor parallel per batch
HPC = H // TPW              # heads per core = 4
C = HPC * DH                # channels per core = 256
SCALE = 1.0 / 8.0           # 1/sqrt(DH)
VW = HPC * (DH + 1)         # V storage width per s-tile (ones col per head)
NST = S // 128              # 16 s-tiles of 128 rows
NQB = S // 512              # 4 q-blocks of 512
NEC = E // 128              # 8 e-chunks (contraction for projections)

_cache = {}


def _emit(nc, tc, causal):
    # ---- DRAM parameters ----
    xt_d = nc.dram_tensor("xt", [E, S], MM_DT, kind="ExternalInput").ap()
    # weights arrive pre-packed in SBUF layout: [partition, ec*C + c]
    wqt_d = nc.dram_tensor("wqt", [128, NEC * C], MM_DT, kind="ExternalInput").ap()
    wkt_d = nc.dram_tensor("wkt", [128, NEC * C], MM_DT, kind="ExternalInput").ap()
    wvt_d = nc.dram_tensor("wvt", [128, NEC * C], MM_DT, kind="ExternalInput").ap()
    wot_d = nc.dram_tensor("wot", [128, 2 * E], MM_DT, kind="ExternalInput").ap()
    bqk_d = nc.dram_tensor("bqk", [128, 4], F32, kind="ExternalInput").ap()
    bv_d = nc.dram_tensor("bv", [1, C], F32, kind="ExternalInput").ap()
    bo_d = nc.dram_tensor("bo", [1, E], F32, kind="ExternalInput").ap()
    ones_d = nc.dram_tensor("ones", [1, 128], F32, kind="ExternalInput").ap()
    onesv_d = nc.dram_tensor("onesv", [128, NST * HPC], F32, kind="ExternalInput").ap()
    out_d = nc.dram_tensor("out", [S, E], BF16, kind="ExternalOutput").ap()

    ctxpool = tc.tile_pool

    with ctxpool(name="persist", bufs=1) as pp:
        # ---- persistent SBUF tensors ----
        # per-e-chunk X^T tiles and half-tensor weight tiles: separate
        # tiles keep DMA-write -> matmul-read dependencies chunk-granular
        # (reads of one big tile can end up waiting on ALL its DMA writes)
        xts = [pp.tile([128, S], MM_DT, name=f"xt{ec}") for ec in range(NEC)]
        wvts = [pp.tile([128, NEC * C // 2], MM_DT, name=f"wvt{i}")
                for i in range(2)]
        wots = [pp.tile([128, E], MM_DT, name=f"wot{cc}") for cc in range(2)]
        qt_sb = pp.tile([128, 2 * S], MM_DT)         # Q^T, d-tile t at cols [t*S)
        kt_sb = pp.tile([128, 2 * S], MM_DT)
        v_sb = pp.tile([128, NST * VW], MM_DT)       # V (+ones col per head)
        ot_sb = pp.tile([128, 2 * S], MM_DT)         # normalized attn out^T
        bqk_sb = pp.tile([128, 4], F32)
        bvb_sb = pp.tile([128, C], F32)             # bv broadcast to partitions
        bob_sb = pp.tile([128, E], F32)             # bo broadcast to partitions
        ones_r = pp.tile([1, 128], F32R)

        def emit_vproj(psum_pool, st, vtag="mps"):
            """Project V for s-tile st into v_sb (with per-head ones column)."""
            ps = psum_pool.tile([128, C], F32, tag=vtag, name=f"vp{st}")
            for ec in range(NEC):
                nc.tensor.matmul(
                    ps[:],
                    xts[ec][:, st * 128: st * 128 + 128],
                    wv_ap(ec),
                    start=(ec == 0), stop=(ec == NEC - 1),
                    skip_group_check=True)
            dst = v_sb[:, st * VW: st * VW + VW].rearrange(
                "p (h x) -> p h x", h=HPC)[:, :, 0:DH]
            nc.vector.tensor_add(
                dst,
                ps[:].rearrange("p (h x) -> p h x", h=HPC),
                bvb_sb[:].rearrange("p (h x) -> p h x", h=HPC))

        with ctxpool(name="qkw", bufs=1) as qkw, \
             ctxpool(name="small", bufs=1) as sp:
            wqts = [qkw.tile([128, NEC * C // 2], MM_DT, name=f"wqt{i}")
                    for i in range(2)]
            wkts = [qkw.tile([128, NEC * C // 2], MM_DT, name=f"wkt{i}")
                    for i in range(2)]
            HC = NEC * C // 2

            def wq_ap(ec, c0, c1):
                return wqts[ec // 4][:, (ec % 4) * C + c0:(ec % 4) * C + c1]

            def wk_ap(ec, c0, c1):
                return wkts[ec // 4][:, (ec % 4) * C + c0:(ec % 4) * C + c1]

            def wv_ap(ec):
                return wvts[ec // 4][:, (ec % 4) * C:(ec % 4) * C + C]

            bv_row = sp.tile([1, C], F32R)
            bo_row = sp.tile([1, E], F32R)
            onesb_sb = sp.tile([128, NST * HPC], F32)

            # ---- input DMAs: consumption order, round-robin across the
            # three DMA-capable rings (sync/scalar/gpsimd). DMA cost is
            # descriptor-bound (~128 descriptors per transfer regardless of
            # size), so each DMA moves big contiguous per-partition chunks
            # (2-4KB/partition) and aggregate ring throughput is what counts.
            rings = [nc.sync, nc.scalar, nc.gpsimd]
            ric = [0]

            def dma(out, in_):
                rings[ric[0] % 3].dma_start(out=out, in_=in_)
                ric[0] += 1

            # tiny transfers first: one per ring to absorb any first-DMA
            # warmup before the critical weight/activation chunks
            half = NEC * C // 2
            dma(ones_r[:], ones_d[:].bitcast(F32R))
            dma(bv_row[:], bv_d[:].bitcast(F32R))
            dma(bo_row[:], bo_d[:].bitcast(F32R))
            dma(wqts[0][:], wqt_d[:, 0:half])
            dma(wkts[0][:], wkt_d[:, 0:half])
            dma(xts[0][:], xt_d[0:128, :])
            dma(xts[1][:], xt_d[128:256, :])
            dma(xts[2][:], xt_d[256:384, :])
            dma(wqts[1][:], wqt_d[:, half:])
            dma(wkts[1][:], wkt_d[:, half:])
            for ec in range(3, NEC):
                dma(xts[ec][:], xt_d[ec * 128:(ec + 1) * 128, :])
            dma(bqk_sb[:], bqk_d[:])
            dma(onesb_sb[:], onesv_d[:])
            dma(wvts[0][:], wvt_d[:, 0:half])
            dma(wvts[1][:], wvt_d[:, half:])
            dma(wots[0][:], wot_d[:, 0:E])
            dma(wots[1][:], wot_d[:, E:])
            # V ones columns via a strided DVE copy (a strided DMA here costs
            # ~10us of descriptor generation and blocks the ring)
            v_ones_ap = v_sb[:].rearrange("p (n x) -> p n x", x=DH + 1)[:, :, DH:DH + 1]
            nc.vector.tensor_copy(
                v_ones_ap, onesb_sb[:].rearrange("p (n x) -> p n x", x=1))

            # ==== phase B: Q^T/K^T projections (e-chunk outer, 8 live
            # accumulation groups; PE paced by the DMA stream) ====
            with ctxpool(name="proj_ps", bufs=8, space="PSUM") as proj_ps:
                # bias tiles allocated first (ring slots ahead of the live
                # dt0 accumulators) but their matmuls are emitted after the
                # first e-chunk so PE doesn't stall on the small DMAs
                ps_bv = proj_ps.tile([128, C], F32, tag="pps")
                ps_bos = [proj_ps.tile([128, 512], F32, tag="pps",
                                       name=f"bo{eb}") for eb in range(2)]
                # dt0: e-chunk outer, 8 live groups -> PE paced by DMA arrival
                pss = {}
                for pj in range(2):
                    for sb_i in range(NQB):
                        pss[pj, sb_i] = proj_ps.tile(
                            [128, 512], F32, tag="pps",
                            name=f"pp_0_{pj}_{sb_i}")
                # the last e-chunk runs in drain order (matching add_order
                # below) so the DVE adds overlap the remaining ec7 matmuls
                add_order = [(1, 1), (1, 2), (0, 0), (1, 0),
                             (1, 3), (0, 1), (0, 2), (0, 3)]
                for ec in range(NEC):
                    pairs = (add_order if ec == NEC - 1 else
                             [(pj, sb_i) for pj in range(2)
                              for sb_i in range(NQB)])
                    for pj, sb_i in pairs:
                        w_ap = (wq_ap if pj == 0 else wk_ap)(ec, 0, 128)
                        nc.tensor.matmul(
                            pss[pj, sb_i][:],
                            w_ap,
                            xts[ec][:, sb_i * 512: sb_i * 512 + 512],
                            start=(ec == 0), stop=(ec == NEC - 1),
                            skip_group_check=True)
                    if ec == 1:
                        # bias broadcasts via K=1 fp32 matmul against ones;
                        # copies on ACT (idle here) to keep DVE clear
                        nc.tensor.matmul(ps_bv[:], ones_r[0:1, 0:128],
                                         bv_row[:], start=True, stop=True)
                        nc.scalar.activation(bvb_sb[:], ps_bv[:], ActF.Copy)
                        for eb in range(2):
                            nc.tensor.matmul(ps_bos[eb][:], ones_r[0:1, 0:128],
                                             bo_row[0:1, eb * 512:(eb + 1) * 512],
                                             start=True, stop=True)
                            nc.scalar.activation(
                                bob_sb[:, eb * 512:(eb + 1) * 512],
                                ps_bos[eb][:], ActF.Copy)
                # phase C's first score tiles reuse the PSUM banks behind the
                # late ring slots (pss[1,1..3]), so drain those first, then
                # sb0 (needed by the first scores)
                for pj, sb_i in add_order:
                    o_sb, bcol = (qt_sb, 0) if pj == 0 else (kt_sb, 2)
                    nc.vector.tensor_scalar_add(
                        o_sb[:, sb_i * 512: sb_i * 512 + 512],
                        pss[pj, sb_i][:],
                        bqk_sb[:, bcol: bcol + 1])
                # non-causal: every q-block reads every k-tile, so V must be
                # fully projected up front (causal defers st0-3 to phase C)
                if not causal:
                    for st in range(NST):
                        emit_vproj(proj_ps, st, vtag="pps")

            # ==== phase C: attention (q-block outer, head inner) + out-proj ====
            # attention PSUM pools live in their own ExitStack so they can be
            # closed before the tail (their banks back the tail's deep ring)
            with contextlib.ExitStack() as apools, \
                 ctxpool(name="pt_pool", bufs=10) as pt_pool, \
                 ctxpool(name="rec_pool", bufs=4) as rec_pool, \
                 ctxpool(name="bc_pool", bufs=4) as bc_pool, \
                 ctxpool(name="out_pool", bufs=8) as out_pool:
                score_ps = apools.enter_context(
                    ctxpool(name="score_ps", bufs=2, space="PSUM"))
                attn_ps = apools.enter_context(
                    ctxpool(name="attn_ps", bufs=2, space="PSUM"))
                misc_ps = apools.enter_context(
                    ctxpool(name="misc_ps", bufs=2, space="PSUM"))
                def emit_dt1():
                    # second-d-tile Q/K projections: overlap the first
                    # q-block's attention; gate only head pair 1
                    for sb_i in range(NQB):
                        for pj, w_fn, o_sb, bcol in ((0, wq_ap, qt_sb, 0),
                                                     (1, wk_ap, kt_sb, 2)):
                            ps1 = misc_ps.tile([128, 512], F32, tag="mps",
                                               name=f"pp1_{pj}_{sb_i}")
                            for ec in range(NEC):
                                nc.tensor.matmul(
                                    ps1[:],
                                    w_fn(ec, 128, 256),
                                    xts[ec][:, sb_i * 512:
                                            sb_i * 512 + 512],
                                    start=(ec == 0), stop=(ec == NEC - 1),
                                    skip_group_check=True)
                            nc.vector.tensor_scalar_add(
                                o_sb[:, S + sb_i * 512: S + sb_i * 512 + 512],
                                ps1[:], bqk_sb[:, bcol + 1: bcol + 2])

                out_tiles = {}
                pending = []   # deferred norm closures of the previous hp
                pending_f = []  # deferred filler closures (outproj/vproj)

                def flush_pending():
                    while pending:
                        pending.pop(0)()

                def flush_fillers(all_=False):
                    if pending_f:
                        pending_f.pop(0)()
                    while all_ and pending_f:
                        pending_f.pop(0)()

                def emit_outproj_half(qb, eb, vproj_first=False):
                    # out-projection of q-block qb, output columns
                    # [eb*512, eb*512+512): both c-chunks accumulate in PSUM,
                    # one DVE bias-add per tile; store on the eb=1 half
                    with tc.high_priority(offset=-1_000_000):
                        if vproj_first and causal and qb + 1 < NQB:
                            for st in range(4 * (qb + 1), 4 * (qb + 2)):
                                emit_vproj(misc_ps, st)
                        for st in range(qb * 4, qb * 4 + 4):
                            if eb == 0:
                                o_t = out_pool.tile([128, E], BF16, tag="ob",
                                                    name=f"ot{st}")
                                out_tiles[st] = o_t
                            else:
                                o_t = out_tiles[st]
                            ps_f = misc_ps.tile([128, 512], F32, tag="mps",
                                                name=f"pg{st}{eb}")
                            nc.tensor.matmul(
                                ps_f[:],
                                ot_sb[:, st * 128: st * 128 + 128],
                                wots[0][:, eb * 512: eb * 512 + 512],
                                start=True, stop=False,
                                skip_group_check=True)
                            nc.tensor.matmul(
                                ps_f[:],
                                ot_sb[:, S + st * 128: S + st * 128 + 128],
                                wots[1][:, eb * 512: eb * 512 + 512],
                                start=False, stop=True,
                                skip_group_check=True)
                            nc.vector.tensor_add(
                                o_t[:, eb * 512:(eb + 1) * 512], ps_f[:],
                                bob_sb[:, eb * 512:(eb + 1) * 512])
                            if eb == 1:
                                # sync ring: the scalar engine is busy with
                                # exp during the steady state
                                nc.sync.dma_start(
                                    out=out_d[st * 128:(st + 1) * 128, :],
                                    in_=o_t[:])

                def emit_outproj_cc0(qb):
                    # last q-block: first-half out-projection (heads 0,1) at
                    # normal priority so it fills the ACT-wait gaps of the
                    # last (and longest) attention block
                    for st in range(qb * 4, qb * 4 + 4):
                        o_t = out_pool.tile([128, E], BF16, tag="ob",
                                            name=f"ot{st}")
                        out_tiles[st] = o_t
                        for eb in range(2):
                            ps_f = misc_ps.tile([128, 512], F32, tag="mps",
                                                name=f"pg{st}{eb}")
                            nc.tensor.matmul(
                                ps_f[:],
                                ot_sb[:, st * 128: st * 128 + 128],
                                wots[0][:, eb * 512: eb * 512 + 512],
                                start=True, stop=True)
                            nc.vector.tensor_add(
                                o_t[:, eb * 512:(eb + 1) * 512], ps_f[:],
                                bob_sb[:, eb * 512:(eb + 1) * 512])

                def emit_outproj_cc1_last(qb, tail_ps):
                    # last q-block tail: heads 2,3 half + store; deep PSUM
                    # ring (banks freed by the closed attention pools) keeps
                    # the matmul stream ahead of the DVE adds; stores
                    # alternate between the sync and scalar rings
                    for st in range(qb * 4, qb * 4 + 4):
                        o_t = out_tiles[st]
                        for eb in range(2):
                            ps_f = tail_ps.tile([128, 512], F32, tag="tps",
                                                name=f"pf{st}{eb}")
                            nc.tensor.matmul(
                                ps_f[:],
                                ot_sb[:, S + st * 128: S + st * 128 + 128],
                                wots[1][:, eb * 512: eb * 512 + 512],
                                start=True, stop=True)
                            nc.vector.tensor_add(
                                o_t[:, eb * 512:(eb + 1) * 512], ps_f[:],
                                o_t[:, eb * 512:(eb + 1) * 512])
                            eng = nc.sync if eb == 0 else nc.scalar
                            eng.dma_start(
                                out=out_d[st * 128:(st + 1) * 128,
                                          eb * 512:(eb + 1) * 512],
                                in_=o_t[:, eb * 512:(eb + 1) * 512])

                for qb in range(NQB):
                    nk = 4 * (qb + 1) if causal else NST
                    q0 = qb * 512
                    for hp in range(2):   # head pair (2*hp, 2*hp+1), d-tile hp
                        t = hp
                        ps_os = [None, None]

                        def emit_pv(kt_i, pt, hp=hp, nk=nk, qb=qb):
                            if kt_i == 0:
                                for a in range(2):
                                    ps_os[a] = attn_ps.tile(
                                        [65, 512], F32, tag="po",
                                        name=f"po{qb}{hp}{a}")
                            # causal: column group j (128 q-cols) receives
                            # its last contribution at diagonal tile
                            # kt=4qb+j, so diagonal tiles stop their own
                            # group and skip the fully-masked left columns
                            off = kt_i * 128 - qb * 512
                            j0 = off // 128 if (causal and off >= 0) else None
                            for a in range(2):
                                h = 2 * hp + a
                                vsl = v_sb[:, kt_i * VW + h * (DH + 1):
                                           kt_i * VW + h * (DH + 1) + DH + 1]
                                if j0 is None:
                                    nc.tensor.matmul(
                                        ps_os[a][:], vsl,
                                        pt[:, a * 512:(a + 1) * 512],
                                        start=(kt_i == 0),
                                        stop=(not causal and kt_i == nk - 1),
                                        skip_group_check=True)
                                else:
                                    c0 = 128 * j0
                                    nc.tensor.matmul(
                                        ps_os[a][:, c0:c0 + 128], vsl,
                                        pt[:, a * 512 + c0:
                                           a * 512 + c0 + 128],
                                        start=(kt_i == 0), stop=True,
                                        skip_group_check=True)
                                    if j0 < 3:
                                        nc.tensor.matmul(
                                            ps_os[a][:, c0 + 128:512], vsl,
                                            pt[:, a * 512 + c0 + 128:
                                               (a + 1) * 512],
                                            start=(kt_i == 0), stop=False,
                                            skip_group_check=True)

                        pv_queue = []
                        for kt_i in range(nk):
                            # c0: fully-masked left columns of diagonal tiles
                            off = kt_i * 128 - q0
                            c0 = max(0, off) if causal else 0
                            ps_s = score_ps.tile([128, 1024], F32, tag="sc",
                                                 name=f"sc{qb}{hp}{kt_i}")
                            pt = pt_pool.tile([128, 1024], MM_DT, tag="pt",
                                              name=f"pt{qb}{hp}{kt_i}")
                            # the two heads' score matmuls target different PE
                            # row-groups (rows 0-63 vs 64-127) -> run conc.
                            for a in range(2):
                                p0 = a * 64
                                nc.tensor.matmul(
                                    ps_s[:, a * 512 + c0:(a + 1) * 512],
                                    kt_sb[p0:p0 + 64,
                                          t * S + kt_i * 128: t * S + kt_i * 128 + 128],
                                    qt_sb[p0:p0 + 64,
                                          t * S + q0 + c0: t * S + q0 + 512],
                                    start=True, stop=True)
                            if c0 == 0:
                                nc.scalar.activation(pt[:], ps_s[:], ActF.Exp,
                                                     scale=SCALE)
                            else:
                                for a in range(2):
                                    nc.scalar.activation(
                                        pt[:, a * 512 + c0:(a + 1) * 512],
                                        ps_s[:, a * 512 + c0:(a + 1) * 512],
                                        ActF.Exp, scale=SCALE)
                            if causal and off + 127 >= 0:
                                # zero masked left cols + diagonal triangle;
                                # cols right of the triangle stay untouched
                                c1 = min(c0 + 128, 512)
                                view = pt[:].rearrange(
                                    "p (u q) -> p u q", u=2)[:, :, 0:c1]
                                nc.gpsimd.affine_select(
                                    out=view, in_=view,
                                    compare_op=Alu.is_ge,
                                    fill=0.0, base=-off,
                                    pattern=[[0, 2], [1, c1]],
                                    channel_multiplier=-1)
                            if causal and qb == 0 and hp == 0 and kt_i == 1:
                                # first q-block's V projection: emitted after
                                # the first scores/exp so the ACT pipeline
                                # starts as early as possible (PV(kt0) is
                                # emitted later this iteration and waits on
                                # these via the v_sb dependency)
                                for st in range(4):
                                    emit_vproj(misc_ps, st)
                            if kt_i == 0:
                                # previous hp's norms land here, after this
                                # hp's first scores/exp are in the stream
                                flush_pending()
                            if kt_i == min(2, nk - 1):
                                # fillers later still, clear of the boundary
                                flush_fillers(
                                    all_=(qb == NQB - 1 and hp == 1))
                            # defer this step's PV by two steps: covers the
                            # exp latency (~1.1us vs ~0.65us of scores per
                            # k-tile) so PV never waits on ACT mid-stream
                            pv_queue.append((kt_i, pt))
                            if len(pv_queue) > 2:
                                emit_pv(*pv_queue.pop(0))
                        while pv_queue:
                            emit_pv(*pv_queue.pop(0))

                        def norm(qb=qb, hp=hp, t=t, q0=q0, ps_os=ps_os):
                            for a in range(2):
                                h = 2 * hp + a
                                p0 = a * 64
                                rs = rec_pool.tile([1, 512], F32R, tag="rs",
                                                   name=f"rs{qb}{h}")
                                nc.vector.tensor_copy(rs[:], ps_os[a][64:65, :])
                                ps_b = misc_ps.tile([64, 512], F32, tag="mps",
                                                    name=f"pb{qb}{h}")
                                nc.tensor.matmul(ps_b[:], ones_r[0:1, 0:64],
                                                 rs[:], start=True, stop=True)
                                bc = bc_pool.tile([64, 512], F32, tag="bc",
                                                  name=f"bc{qb}{h}")
                                nc.vector.reciprocal_approx_fast(bc[:], ps_b[:])
                                nc.vector.tensor_mul(
                                    ot_sb[p0:p0 + 64,
                                          t * S + q0: t * S + q0 + 512],
                                    ps_os[a][0:64, :], bc[:])
                        if qb == 0 and hp == 0:
                            emit_dt1()
                        pending.append(norm)
                        if hp == 1 and qb < NQB - 1:
                            pending_f.append(
                                lambda qb=qb: emit_outproj_half(
                                    qb, 0, vproj_first=True))
                            pending_f.append(
                                lambda qb=qb: emit_outproj_half(qb, 1))
                        elif hp == 0 and qb == NQB - 1:
                            pending_f.append(
                                lambda qb=qb: emit_outproj_cc0(qb))
                flush_pending()
                flush_fillers(all_=True)
                # close attention PSUM pools; their banks back the tail ring
                apools.close()
                with ctxpool(name="tail_ps", bufs=4, space="PSUM") as tail_ps:
                    emit_outproj_cc1_last(NQB - 1, tail_ps)

def _build(causal):
    nc = bacc.Bacc("TRN2", target_bir_lowering=False, debug=False,
                   num_devices=NCORES)
    with tile.TileContext(nc) as tc:
        _emit(nc, tc, causal)
    nc.compile()
    return nc


def _shard_inputs(QKV, Wq, bq, Wk, bk, Wv, bv, Wo, bo):
    QKV = np.asarray(QKV, dtype=np.float32)
    Wq, Wk, Wv, Wo = (np.asarray(w, dtype=np.float32) for w in (Wq, Wk, Wv, Wo))
    bq, bk, bv, bo = (np.asarray(b_, dtype=np.float32) for b_ in (bq, bk, bv, bo))
    ones = np.ones((1, 128), dtype=np.float32)
    onesv = np.ones((128, NST * HPC), dtype=np.float32)

    def pack(wt):
        # [n*128, m] -> [128, n*m] in SBUF layout (partition, chunk*m + col)
        n = wt.shape[0] // 128
        return np.ascontiguousarray(
            wt.reshape(n, 128, -1).transpose(1, 0, 2).reshape(128, -1)
        ).astype(MM_NP)

    in_maps = []
    for core in range(NCORES):
        b, g = divmod(core, TPW)
        cs = slice(g * C, (g + 1) * C)
        bqs, bks = bq[cs], bk[cs]
        bqk = np.stack([bqs[:128], bqs[128:], bks[:128], bks[128:]], axis=1)
        in_maps.append({
            "xt": np.ascontiguousarray(QKV[b].T).astype(MM_NP),
            "wqt": pack(Wq[cs, :].T),
            "wkt": pack(Wk[cs, :].T),
            "wvt": pack(Wv[cs, :].T),
            "wot": pack(Wo[:, cs].T),
            "bqk": np.ascontiguousarray(bqk),
            "bv": bv[cs].reshape(1, C).copy(),
            # host sums the 4 tensor-parallel partials per batch; only one
            # core per group contributes the output bias
            "bo": (bo if g == 0 else np.zeros_like(bo)).reshape(1, E).copy(),
            "ones": ones,
            "onesv": onesv,
        })
    return in_maps


def kernel(QKV, Wq, bq, Wk, bk, Wv, bv, Wo, bo, is_causal):
    causal = bool(int(np.asarray(is_causal)))
    if causal not in _cache:
        _cache[causal] = _build(causal)
    nc = _cache[causal]
    in_maps = _shard_inputs(QKV, Wq, bq, Wk, bk, Wv, bv, Wo, bo)
    res = run_bass_kernel_spmd(nc, in_maps, core_ids=list(range(NCORES)))
    out = np.empty((B, S, E), dtype=np.float32)
    for b in range(B):
        acc = res.results[TPW * b]["out"].astype(np.float32)
        for g in range(1, TPW):
            acc = acc + res.results[TPW * b + g]["out"]
        out[b] = acc
    return out
